# revision 1
# baseline (speedup 1.0000x reference)
"""Multi-head latent attention (MLA) TRN2 kernel.

Sharding: batch(2) x query-sequence(4) over 8 cores. Each core:
  - computes the full KV path for its batch (kv_a, rmsnorm, kv_b, rope)
  - computes the Q path for its 512-token query chunk
  - full attention for its 512 queries x 2048 keys x 16 heads
  - o_proj for its chunk -> output slice [512, 2048]
Host assembles the 8 slices into [B, T, HID]. No collectives.

All matmuls run in float32r (fp32 with 11-bit mantissa, 1 cycle/row on the
PE when N>=256 -- same throughput as bf16 at ~2^-12 relative precision).
Activations are kept feature-major ([feature, token]) so weight matrices act
as lhsT directly as stored; attention computes scores transposed
(s^T[k,q] = k^T q) so softmax needs no transposes: exp on ACT, the
denominator via an all-ones lhsT matmul (broadcast into all 128 partitions),
and P@V consumes the transposed probabilities directly.
"""

import math

import numpy as np

B, T, HID = 2, 2048, 2048
NH, NKV = 16, 8
NOPE, ROPE = 128, 64
HD = NOPE + ROPE  # 192
VD = 128
KV_RANK, Q_RANK = 512, 1536
EPS = 1e-6
THETA = 10000.0
NCORES = 8
TQ = B * T // NCORES  # 512 query tokens per core
P = 128
SCALE = 1.0 / math.sqrt(HD)

# Rope rows are stored "paired": each head's rotated rope halves (32+32 rows)
# are stacked into one contiguous 64-row slot, two heads per 128-partition
# tile, at base partition 64*(kvh%2) so score-matmul lhsT(k)/rhs(q) base
# partitions match (PE only allows bases {0, 32, 64}).

_CACHE = {}


def _round_f32r(a):
    a = np.ascontiguousarray(np.asarray(a, dtype=np.float32))
    u = a.view(np.uint32)
    low = u & np.uint32(0xFFF)
    rounded = u & np.uint32(0xFFFFF000)
    lsb = (u >> np.uint32(12)) & np.uint32(1)
    round_up = (low > 0x800) | ((low == 0x800) & (lsb == 1))
    return (rounded + (round_up.astype(np.uint32) << np.uint32(12))).view(np.float32)


def _build_nc():
    import concourse.bass as bass  # noqa: F401
    import concourse.mybir as mybir
    from concourse import bacc
    from concourse.tile import TileContext

    F32 = mybir.dt.float32
    F32R = mybir.dt.float32r
    AF = mybir.ActivationFunctionType
    ALU = mybir.AluOpType

    nc = bacc.Bacc(None, target_bir_lowering=False)

    xT = nc.dram_tensor("xT", [HID, T], F32R, kind="ExternalInput")
    xq = nc.dram_tensor("xq", [HID, TQ], F32R, kind="ExternalInput")
    qa_w = nc.dram_tensor("qa_w", [HID, Q_RANK], F32R, kind="ExternalInput")
    qa_ln = nc.dram_tensor("qa_ln", [P, Q_RANK // P], F32R, kind="ExternalInput")
    qb_w = nc.dram_tensor("qb_w", [Q_RANK, NH * HD], F32R, kind="ExternalInput")
    kva_w = nc.dram_tensor("kva_w", [HID, KV_RANK + NKV * ROPE], F32R, kind="ExternalInput")
    kva_ln = nc.dram_tensor("kva_ln", [P, KV_RANK // P], F32R, kind="ExternalInput")
    kvb_w = nc.dram_tensor("kvb_w", [KV_RANK, NKV * (NOPE + VD)], F32R, kind="ExternalInput")
    o_w = nc.dram_tensor("o_w", [NH * VD, HID], F32R, kind="ExternalInput")
    cosq = nc.dram_tensor("cosq", [P, TQ], F32R, kind="ExternalInput")
    sinq = nc.dram_tensor("sinq", [P, TQ], F32R, kind="ExternalInput")
    cosk = nc.dram_tensor("cosk", [P, T], F32R, kind="ExternalInput")
    sink = nc.dram_tensor("sink", [P, T], F32R, kind="ExternalInput")
    ones_in = nc.dram_tensor("ones_in", [P, P], F32R, kind="ExternalInput")
    eps_in = nc.dram_tensor("eps_in", [P, 2], F32, kind="ExternalInput")
    out = nc.dram_tensor("out", [TQ, HID], F32, kind="ExternalOutput")

    xT_t = xT.rearrange("(kt p) t -> p kt t", p=P)  # [128, 16, T]
    xq_t = xq.rearrange("(kt p) t -> p kt t", p=P)  # [128, 16, TQ]

    with TileContext(nc) as tc:
        with (
            tc.tile_pool(name="tables", bufs=1) as tbl,
            tc.tile_pool(name="dram", bufs=1, space="DRAM") as dpool,
            tc.tile_pool(name="pAttn", bufs=1) as pAttn,
        ):
            ones_sb = tbl.tile([P, P], F32R, name="ones_sb")
            nc.sync.dma_start(ones_sb[:], ones_in[:, :])
            lnq_sb = tbl.tile([P, Q_RANK // P], F32R, name="lnq_sb")
            nc.sync.dma_start(lnq_sb[:], qa_ln[:, :])
            lnkv_sb = tbl.tile([P, KV_RANK // P], F32R, name="lnkv_sb")
            nc.sync.dma_start(lnkv_sb[:], kva_ln[:, :])
            eps_sb = tbl.tile([P, 2], F32, name="eps_sb")
            nc.sync.dma_start(eps_sb[:], eps_in[:, :])
            epskv_sb = eps_sb[:, 0:1]
            epsq_sb = eps_sb[:, 1:2]

            kpaird = dpool.tile([P, 4, T], F32R, name="kpaird")
            qnoped = dpool.tile([P, NH, TQ], F32R, name="qnoped")
            qpaird = dpool.tile([P, 8, TQ], F32R, name="qpaird")

            # attention output, resident through P3+P4
            attn_sb = pAttn.tile([P, NH, TQ], F32R, name="attn_sb")

            with tc.tile_pool(name="pLat", bufs=1) as pLat:
                kv_latN = pLat.tile([P, 4, T], F32R, name="kv_latN")

                # ------------- P2: q path (first; no kv deps) ---------------
                with (
                    tc.tile_pool(name="p2", bufs=1) as p2,
                    tc.tile_pool(name="p2s", bufs=2) as p2s,
                    tc.tile_pool(name="p2w", bufs=3) as p2w,
                    tc.tile_pool(name="p2ps", bufs=2, space="PSUM") as p2ps,
                    tc.tile_pool(name="p2ps1", bufs=1, space="PSUM") as p2ps1,
                ):
                    q_lat = p2.tile([P, Q_RANK // P, TQ], F32R, name="q_lat")
                    rs_q = p2.tile([P, TQ], F32, name="rs_q")

                    with tc.tile_pool(name="p2xq", bufs=1) as p2xq:
                        xq_c = []
                        for c in range(4):
                            t_ = p2xq.tile([P, 4, TQ], F32R, name=f"xq_c{c}")
                            nc.sync.dma_start(t_[:], xq_t[:, 4 * c : 4 * c + 4, :])
                            xq_c.append(t_)

                        # q_a + rmsnorm
                        sumsq = p2ps1.tile([P, TQ], F32, tag="qsumsq")
                        for m in range(12):
                            wt = p2w.tile([P, 16, P], F32R, tag="qa_wt")
                            nc.sync.dma_start(
                                wt[:],
                                qa_w.rearrange("(kt p) c -> p kt c", p=P)[
                                    :, :, m * P : (m + 1) * P
                                ],
                            )
                            ps = p2ps.tile([P, TQ], F32, tag="qa_ps")
                            for k in range(16):
                                nc.tensor.matmul(
                                    ps[:], wt[:, k, :], xq_c[k // 4][:, k % 4, :],
                                    start=(k == 0), stop=(k == 15),
                                )
                            nc.vector.tensor_copy(q_lat[:, m, :], ps[:])
                            sq = p2s.tile([P, TQ], F32R, tag="qsq")
                            nc.scalar.square(sq[:], ps[:])
                            nc.tensor.matmul(
                                sumsq[:], ones_sb[:], sq[:],
                                start=(m == 0), stop=(m == 11),
                            )
                        sqt = p2s.tile([P, TQ], F32, tag="qsqt")
                        nc.scalar.activation(sqt[:], sumsq[:], AF.Sqrt, bias=epsq_sb[:])
                        nc.vector.reciprocal(rs_q[:], sqt[:])
                        for m in range(Q_RANK // P):
                            nc.vector.scalar_tensor_tensor(
                                q_lat[:, m, :], q_lat[:, m, :],
                                lnq_sb[:, m : m + 1], rs_q[:],
                                ALU.mult, ALU.mult,
                            )

                    # q_b: nope tiles spill to HBM; rope raw kept for rotation
                    with tc.tile_pool(name="p2b", bufs=1) as p2b:
                        qraw1 = p2b.tile([P, 4, TQ], F32R, name="qraw1")
                        qraw2 = p2b.tile([P, 4, TQ], F32R, name="qraw2")
                        for m in range(24):
                            wt = p2w.tile([P, 12, P], F32R, tag="qb_wt")
                            nc.sync.dma_start(
                                wt[:],
                                qb_w.rearrange("(kt p) c -> p kt c", p=P)[
                                    :, :, m * P : (m + 1) * P
                                ],
                            )
                            ps = p2ps.tile([P, TQ], F32, tag="qb_ps")
                            for k in range(12):
                                nc.tensor.matmul(
                                    ps[:], wt[:, k, :], q_lat[:, k, :],
                                    start=(k == 0), stop=(k == 11),
                                )
                            if m < 16:
                                st = p2s.tile([P, TQ], F32R, tag="qn_st")
                                nc.scalar.copy(st[:], ps[:])
                                nc.sync.dma_start(qnoped[:, m, :], st[:])
                            elif m < 20:
                                nc.scalar.copy(qraw1[:, m - 16, :], ps[:])
                            else:
                                nc.scalar.copy(qraw2[:, m - 20, :], ps[:])

                        # q-rope rotation then scatter to paired HBM layout
                        cosq_sb = p2b.tile([P, TQ], F32R, name="cosq_sb")
                        nc.sync.dma_start(cosq_sb[:], cosq[:, :])
                        sinq_sb = p2b.tile([P, TQ], F32R, name="sinq_sb")
                        nc.sync.dma_start(sinq_sb[:], sinq[:, :])
                        cb = cosq_sb[:, None, :].to_broadcast((P, 4, TQ))
                        sb = sinq_sb[:, None, :].to_broadcast((P, 4, TQ))
                        qrot1 = p2b.tile([P, 4, TQ], F32R, name="qrot1")
                        qrot2 = p2b.tile([P, 4, TQ], F32R, name="qrot2")
                        tmp = p2b.tile([P, 4, TQ], F32R, name="qrot_tmp1")
                        nc.vector.tensor_tensor(tmp[:], qraw2[:], sb, ALU.mult)
                        nc.vector.tensor_tensor(qrot1[:], qraw1[:], cb, ALU.mult)
                        nc.vector.tensor_tensor(qrot1[:], qrot1[:], tmp[:], ALU.subtract)
                        tmp2 = p2b.tile([P, 4, TQ], F32R, name="qrot_tmp2")
                        nc.vector.tensor_tensor(tmp2[:], qraw1[:], sb, ALU.mult)
                        nc.vector.tensor_tensor(qrot2[:], qraw2[:], cb, ALU.mult)
                        nc.vector.tensor_tensor(qrot2[:], qrot2[:], tmp2[:], ALU.add)
                        # head h -> tile 2*(h//4)+h%2, base 64*((h//2)%2)
                        for h in range(NH):
                            tq_ = 2 * (h // 4) + h % 2
                            bb = 64 * ((h // 2) % 2)
                            nc.sync.dma_start(
                                qpaird[bb : bb + 32, tq_, :],
                                qrot1[(h % 4) * 32 : (h % 4) * 32 + 32, h // 4, :],
                            )
                            nc.sync.dma_start(
                                qpaird[bb + 32 : bb + 64, tq_, :],
                                qrot2[(h % 4) * 32 : (h % 4) * 32 + 32, h // 4, :],
                            )

                # ------------- P1: kv_a + rmsnorm + interleaved rotation ----
                with (
                    tc.tile_pool(name="p1", bufs=1) as p1,
                    tc.tile_pool(name="p1s", bufs=2) as p1s,
                    tc.tile_pool(name="p1ps", bufs=2, space="PSUM") as p1ps,
                    tc.tile_pool(name="p1ps1", bufs=1, space="PSUM") as p1ps1,
                ):
                    kvaw_c = []
                    for c in range(4):
                        t_ = p1.tile([P, 16, 256], F32R, name=f"kvaw_c{c}")
                        nc.sync.dma_start(
                            t_[:],
                            kva_w.rearrange("(kt p) c -> p kt c", p=P)[
                                :, :, c * 256 : (c + 1) * 256
                            ],
                        )
                        kvaw_c.append(t_)

                    def kvaw_at(k, m):
                        return kvaw_c[m // 2][:, k, (m % 2) * P : (m % 2 + 1) * P]

                    cosk_sb = p1.tile([P, T], F32R, name="cosk_sb")
                    nc.sync.dma_start(cosk_sb[:], cosk[:, :])
                    sink_sb = p1.tile([P, T], F32R, name="sink_sb")
                    nc.sync.dma_start(sink_sb[:], sink[:, :])
                    rs_kv = p1.tile([P, 8, 256], F32, name="rs_kv")

                    NCH = 8
                    CW = T // NCH  # 256
                    for nch in range(NCH):
                        chsl = slice(nch * CW, (nch + 1) * CW)
                        xch = p1s.tile([P, 16, CW], F32R, tag="xch")
                        nc.sync.dma_start(xch[:], xT_t[:, :, chsl])
                        sumsq = p1ps1.tile([P, CW], F32, tag="sumsq")
                        raw1 = p1s.tile([P, 2, CW], F32R, tag="kraw1")
                        raw2 = p1s.tile([P, 2, CW], F32R, tag="kraw2")
                        for m in range(8):
                            ps = p1ps.tile([P, CW], F32, tag="kva_ps")
                            for k in range(16):
                                nc.tensor.matmul(
                                    ps[:], kvaw_at(k, m), xch[:, k, :],
                                    start=(k == 0), stop=(k == 15),
                                )
                            if m < 4:
                                nc.vector.tensor_copy(kv_latN[:, m, chsl], ps[:])
                                sq = p1s.tile([P, CW], F32R, tag="sq")
                                nc.scalar.square(sq[:], ps[:])
                                nc.tensor.matmul(
                                    sumsq[:], ones_sb[:], sq[:],
                                    start=(m == 0), stop=(m == 3),
                                )
                            elif m < 6:
                                nc.scalar.copy(raw1[:, m - 4, :], ps[:])
                            else:
                                nc.scalar.copy(raw2[:, m - 6, :], ps[:])
                        sqt = p1s.tile([P, CW], F32, tag="sqt")
                        nc.scalar.activation(sqt[:], sumsq[:], AF.Sqrt, bias=epskv_sb[:])
                        nc.vector.reciprocal(rs_kv[:, nch, :], sqt[:])
                        for m in range(4):
                            nc.vector.scalar_tensor_tensor(
                                kv_latN[:, m, chsl],
                                kv_latN[:, m, chsl],
                                lnkv_sb[:, m : m + 1],
                                rs_kv[:, nch, :],
                                ALU.mult,
                                ALU.mult,
                            )
                        # rotate this chunk's rope rows and scatter to HBM
                        for t in range(2):
                            tmp = p1s.tile([P, CW], F32R, tag="rot_tmp")
                            rot = p1s.tile([P, CW], F32R, tag="rot_out")
                            nc.vector.tensor_tensor(
                                tmp[:], raw2[:, t, :], sink_sb[:, chsl], ALU.mult
                            )
                            nc.vector.tensor_tensor(
                                rot[:], raw1[:, t, :], cosk_sb[:, chsl], ALU.mult
                            )
                            nc.vector.tensor_tensor(rot[:], rot[:], tmp[:], ALU.subtract)
                            tmp2 = p1s.tile([P, CW], F32R, tag="rot_tmp")
                            rot2 = p1s.tile([P, CW], F32R, tag="rot_out")
                            nc.vector.tensor_tensor(
                                tmp2[:], raw1[:, t, :], sink_sb[:, chsl], ALU.mult
                            )
                            nc.vector.tensor_tensor(
                                rot2[:], raw2[:, t, :], cosk_sb[:, chsl], ALU.mult
                            )
                            nc.vector.tensor_tensor(rot2[:], rot2[:], tmp2[:], ALU.add)
                            # head kvh=4t+i -> tile kvh//2, base 64*(kvh%2)
                            for i in range(4):
                                kvh = 4 * t + i
                                bb = 64 * (kvh % 2)
                                nc.sync.dma_start(
                                    kpaird[bb : bb + 32, kvh // 2, chsl],
                                    rot[i * 32 : (i + 1) * 32, :],
                                )
                                nc.sync.dma_start(
                                    kpaird[bb + 32 : bb + 64, kvh // 2, chsl],
                                    rot2[i * 32 : (i + 1) * 32, :],
                                )

                # ------------- P3: attention --------------------------------
                with (
                    tc.tile_pool(name="p3s", bufs=2) as p3s,
                    tc.tile_pool(name="p3q", bufs=4) as p3q,
                    tc.tile_pool(name="p3p", bufs=3) as p3p,
                    tc.tile_pool(name="scps", bufs=3, space="PSUM") as scps,
                    tc.tile_pool(name="atps", bufs=2, space="PSUM") as atps,
                    tc.tile_pool(name="prps", bufs=2, space="PSUM") as prps,
                ):
                    pending = []

                    def finalize(item):
                        dsum, at, qh = item
                        dn = scps.tile([P, TQ], F32, tag="sc")
                        nc.tensor.matmul(
                            dn[:], ones_sb[:], dsum[:], start=True, stop=True
                        )
                        rec = p3q.tile([P, TQ], F32, tag="rec")
                        nc.vector.reciprocal(rec[:], dn[:])
                        nc.vector.tensor_tensor(
                            attn_sb[:, qh, :], at[:], rec[:], ALU.mult
                        )

                    for hp in range(4):  # kv-head pairs
                        kvh0 = 2 * hp
                        wn = p3s.tile([P, 4, 256], F32R, tag="wn")
                        nc.sync.dma_start(
                            wn[:],
                            kvb_w.rearrange("(kt p) c -> p kt c", p=P)[
                                :, :, kvh0 * NOPE : (kvh0 + 2) * NOPE
                            ],
                        )
                        wv = p3s.tile([P, 4, 256], F32R, tag="wv")
                        nc.sync.dma_start(
                            wv[:],
                            kvb_w.rearrange("(kt p) c -> p kt c", p=P)[
                                :, :, NKV * NOPE + kvh0 * VD : NKV * NOPE + (kvh0 + 2) * VD
                            ],
                        )
                        knp = p3s.tile([P, 2, T], F32R, tag="knp")
                        for h2 in range(2):
                            for nch in range(4):
                                ps = prps.tile([P, 512], F32, tag="pr_ps")
                                for k in range(4):
                                    nc.tensor.matmul(
                                        ps[:],
                                        wn[:, k, h2 * P : (h2 + 1) * P],
                                        kv_latN[:, k, nch * 512 : (nch + 1) * 512],
                                        start=(k == 0),
                                        stop=(k == 3),
                                    )
                                nc.vector.tensor_copy(
                                    knp[:, h2, nch * 512 : (nch + 1) * 512], ps[:]
                                )
                        vp = p3s.tile([P, 16, 256], F32R, tag="vp")
                        for mt in range(16):
                            psf = prps.tile([P, 512], F32, tag="pr_ps")
                            ps = psf[:, :256]
                            for k in range(4):
                                nc.tensor.matmul(
                                    ps[:],
                                    kv_latN[:, k, mt * P : (mt + 1) * P],
                                    wv[:, k, :],
                                    start=(k == 0),
                                    stop=(k == 3),
                                )
                            nc.vector.tensor_copy(vp[:, mt, :], ps[:])
                        krp = p3s.tile([P, T], F32R, tag="krp")
                        nc.sync.dma_start(krp[:], kpaird[:, hp, :])
                        qps = {}
                        for tq_ in (2 * hp, 2 * hp + 1):
                            qp = p3q.tile([P, TQ], F32R, tag="qp")
                            nc.sync.dma_start(qp[:], qpaird[:, tq_, :])
                            qps[tq_] = qp

                        for j4 in range(4):
                            qh = 4 * hp + j4
                            kvh = qh // 2
                            h2 = kvh - kvh0
                            b = 64 * (kvh % 2)
                            tq_ = 2 * (qh // 4) + qh % 2
                            qn = p3q.tile([P, TQ], F32R, tag="qn")
                            nc.sync.dma_start(qn[:], qnoped[:, qh, :])
                            qp = qps[tq_]
                            dsum = p3q.tile([P, TQ], F32R, tag="dsum")
                            at = atps.tile([P, TQ], F32, tag="at")
                            pts = {}
                            for kt in range(16):
                                sc = scps.tile([P, TQ], F32, tag="sc")
                                nc.tensor.matmul(
                                    sc[:],
                                    knp[:, h2, kt * P : (kt + 1) * P],
                                    qn[:],
                                    start=True,
                                    stop=False,
                                )
                                nc.tensor.matmul(
                                    sc[:],
                                    krp[b : b + 64, kt * P : (kt + 1) * P],
                                    qp[b : b + 64, :],
                                    start=False,
                                    stop=True,
                                )
                                pt = p3p.tile([P, TQ], F32R, tag="probsT")
                                nc.scalar.activation(
                                    pt[:], sc[:], AF.Exp, scale=float(SCALE)
                                )
                                pts[kt] = pt
                                if kt == 0:
                                    nc.vector.tensor_copy(dsum[:], pt[:])
                                else:
                                    nc.vector.tensor_tensor(
                                        dsum[:], dsum[:], pt[:], ALU.add
                                    )
                                if kt > 0:  # PV one stage behind scores
                                    nc.tensor.matmul(
                                        at[:],
                                        vp[:, kt - 1, h2 * P : (h2 + 1) * P],
                                        pts[kt - 1][:],
                                        start=(kt == 1),
                                        stop=False,
                                    )
                                    del pts[kt - 1]
                            nc.tensor.matmul(
                                at[:],
                                vp[:, 15, h2 * P : (h2 + 1) * P],
                                pts[15][:],
                                start=False,
                                stop=True,
                            )
                            pending.append((dsum, at, qh))
                            if len(pending) == 2:
                                finalize(pending.pop(0))
                    while pending:
                        finalize(pending.pop(0))

            # ------------- P4: o_proj (attn_sb resident) --------------------
            with (
                tc.tile_pool(name="p4s", bufs=2) as p4s,
                tc.tile_pool(name="p4ps", bufs=2, space="PSUM") as p4ps,
            ):
                for n in range(4):
                    ow = p4s.tile([P, 16, 512], F32R, tag="ow")
                    nc.sync.dma_start(
                        ow[:],
                        o_w.rearrange("(ht p) c -> p ht c", p=P)[
                            :, :, n * 512 : (n + 1) * 512
                        ],
                    )
                    for mt in range(4):
                        ps = p4ps.tile([P, 512], F32, tag="o_ps")
                        for h in range(NH):
                            nc.tensor.matmul(
                                ps[:],
                                attn_sb[:, h, mt * P : (mt + 1) * P],
                                ow[:, h, :],
                                start=(h == 0),
                                stop=(h == 15),
                            )
                        st = p4s.tile([P, 512], mybir.dt.float32, tag="ost")
                        nc.scalar.copy(st[:], ps[:])
                        nc.sync.dma_start(
                            out[mt * P : (mt + 1) * P, n * 512 : (n + 1) * 512], st[:]
                        )

    nc.finalize()
    return nc


def _host_prep(inputs):
    r = _round_f32r
    x = np.asarray(inputs["hidden_states"], dtype=np.float32)
    qa_w = r(inputs["q_a_w"])
    qa_ln = r(
        (np.asarray(inputs["q_a_ln_w"], np.float64) * math.sqrt(Q_RANK))
        .astype(np.float32)
        .reshape(Q_RANK // P, P)
        .T.copy()
    )
    kva_ln = r(
        (np.asarray(inputs["kv_a_ln_w"], np.float64) * math.sqrt(KV_RANK))
        .astype(np.float32)
        .reshape(KV_RANK // P, P)
        .T.copy()
    )
    o_w = r(inputs["o_w"])

    qb = np.asarray(inputs["q_b_w"], np.float32).reshape(Q_RANK, NH, HD)
    nope_cols = qb[:, :, :NOPE].reshape(Q_RANK, NH * NOPE)
    rope1 = qb[:, :, NOPE : NOPE + 32].reshape(Q_RANK, 16 * 32)
    rope2 = qb[:, :, NOPE + 32 :].reshape(Q_RANK, 16 * 32)
    qb_w = r(np.concatenate([nope_cols, rope1, rope2], axis=1))

    kva = np.asarray(inputs["kv_a_w"], np.float32)
    lat = kva[:, :KV_RANK]
    krope = kva[:, KV_RANK:].reshape(HID, NKV, ROPE)
    kr1 = krope[:, :, :32].reshape(HID, NKV * 32)
    kr2 = krope[:, :, 32:].reshape(HID, NKV * 32)
    kva_w = r(np.concatenate([lat, kr1, kr2], axis=1))

    kvb = np.asarray(inputs["kv_b_w"], np.float32).reshape(KV_RANK, NKV, NOPE + VD)
    knope_cols = kvb[:, :, :NOPE].reshape(KV_RANK, NKV * NOPE)
    v_cols = kvb[:, :, NOPE:].reshape(KV_RANK, NKV * VD)
    kvb_w = r(np.concatenate([knope_cols, v_cols], axis=1))

    inv_freq = 1.0 / (THETA ** (np.arange(0, ROPE, 2, dtype=np.float32) / ROPE))
    t = np.arange(T, dtype=np.float32)
    freqs = np.outer(t, inv_freq).astype(np.float32)
    cosk = r(np.tile(np.cos(freqs).T, (4, 1)))  # [128, T]
    sink = r(np.tile(np.sin(freqs).T, (4, 1)))
    ones = np.ones((P, P), np.float32)
    eps2 = np.empty((P, 2), np.float32)
    eps2[:, 0] = EPS * KV_RANK
    eps2[:, 1] = EPS * Q_RANK

    in_maps = []
    for c in range(NCORES):
        b, qc = c // 4, c % 4
        xTb = r(x[b].T.copy())
        qoff = qc * TQ
        in_maps.append(
            {
                "xT": xTb,
                "xq": np.ascontiguousarray(xTb[:, qoff : qoff + TQ]),
                "qa_w": qa_w,
                "qa_ln": qa_ln,
                "qb_w": qb_w,
                "kva_w": kva_w,
                "kva_ln": kva_ln,
                "kvb_w": kvb_w,
                "o_w": o_w,
                "cosq": np.ascontiguousarray(cosk[:, qoff : qoff + TQ]),
                "sinq": np.ascontiguousarray(sink[:, qoff : qoff + TQ]),
                "cosk": cosk,
                "sink": sink,
                "ones_in": ones,
                "eps_in": eps2,
            }
        )
    return in_maps


def get_nc():
    if "nc" not in _CACHE:
        _CACHE["nc"] = _build_nc()
    return _CACHE["nc"]


def kernel(**inputs) -> np.ndarray:
    from concourse.bass_utils import run_bass_kernel_spmd

    nc = get_nc()
    in_maps = _host_prep(inputs)
    res = run_bass_kernel_spmd(nc, in_maps, core_ids=list(range(NCORES)))
    _CACHE["last_result"] = res
    outs = [res.results[c]["out"] for c in range(NCORES)]
    full = np.stack(
        [np.concatenate([outs[b * 4 + qc] for qc in range(4)], axis=0) for b in range(B)]
    )
    return full.astype(np.float32)



# revision 53
# speedup vs baseline: 1.5636x; 1.5636x over previous
"""Multi-head latent attention (MLA) TRN2 kernel, v3.

Sharding: batch(2) x query-sequence(4) over 8 cores. Each core:
  - computes kv_a (+rmsnorm+rope) ONLY for its own 512-token chunk and
    AllGathers latent+rope across its 4-core batch group while the whole
    q path runs under the collective
  - computes the Q path for its 512-token query chunk
  - kv_b + full attention for its 512 queries x 2048 keys x 16 heads
  - o_proj for its chunk -> output slice [512, 2048]
Host assembles the 8 slices into [B, T, HID].

Inherited from v2:
  - all matmul operands bf16 (same PE cost as f32r at N>=256, half the
    DMA traffic and SBUF footprint); PSUM stays f32. GPSIMD can't touch
    PSUM and matmuls can't mix f32r/bf16 operands, so the ones-reduction
    tiles (sq, dsA/dsB) are bf16 and all PSUM->SBUF moves sit on ACT/DVE.
  - hidden states arrive rolled so this core's token chunk is at column
    0: kv_a and q_a share the same xq tiles. Key order after the gather
    is the original token order (a permutation, which softmax+PV ignore).
  - q-path rmsnorm scale rs_q commutes through q_b, so it is applied to
    q_b outputs / folded into the rope tables; q_a_ln_w is folded into
    qb_w host-side. Deferred ones-matmuls keep the PE off ACT's tail.
  - rope rotations write 32-row head slices straight into the paired
    layout ([64*(kvh%2)] base so score lhsT/rhs partition bases match).
  - weights prefetched in chunks; gather reload DMAs issue from the
    gpsimd queue so they don't head-of-line-block SP's weight stream.
"""

import math

import numpy as np

B, T, HID = 2, 2048, 2048
NH, NKV = 16, 8
NOPE, ROPE = 128, 64
HD = NOPE + ROPE  # 192
VD = 128
KV_RANK, Q_RANK = 512, 1536
EPS = 1e-6
THETA = 10000.0
NCORES = 8
TQ = B * T // NCORES  # 512 query tokens per core
P = 128
SCALE = 1.0 / math.sqrt(HD)

_CACHE = {}


def _build_nc():
    import concourse.bass as bass  # noqa: F401
    import concourse.mybir as mybir
    from concourse import bacc
    from concourse.tile import TileContext

    F32 = mybir.dt.float32
    BF16 = mybir.dt.bfloat16
    AF = mybir.ActivationFunctionType
    ALU = mybir.AluOpType

    nc = bacc.Bacc(None, target_bir_lowering=False, num_devices=NCORES)

    xT = nc.dram_tensor("xT", [HID, T], BF16, kind="ExternalInput")
    qa_w = nc.dram_tensor("qa_w", [HID, Q_RANK], BF16, kind="ExternalInput")
    qb_w = nc.dram_tensor("qb_w", [Q_RANK, NH * HD], BF16, kind="ExternalInput")
    kva_w = nc.dram_tensor("kva_w", [HID, KV_RANK + NKV * ROPE], BF16, kind="ExternalInput")
    kva_ln = nc.dram_tensor("kva_ln", [P, KV_RANK // P], F32, kind="ExternalInput")
    kvb_w = nc.dram_tensor("kvb_w", [KV_RANK, NKV * (NOPE + VD)], BF16, kind="ExternalInput")
    o_w = nc.dram_tensor("o_w", [NH * VD, HID], BF16, kind="ExternalInput")
    cosq = nc.dram_tensor("cosq", [P, TQ], BF16, kind="ExternalInput")
    sinq = nc.dram_tensor("sinq", [P, TQ], BF16, kind="ExternalInput")
    ones_in = nc.dram_tensor("ones_in", [P, P], BF16, kind="ExternalInput")
    eps_in = nc.dram_tensor("eps_in", [P, 2], F32, kind="ExternalInput")
    out = nc.dram_tensor("out", [TQ, HID], F32, kind="ExternalOutput")

    GROUPS_CC = [[0, 1, 2, 3], [4, 5, 6, 7]]

    xT_t = xT.rearrange("(kt p) t -> p kt t", p=P)  # [128, 16, T]

    with TileContext(nc) as tc:
        with (
            tc.tile_pool(name="tables", bufs=1) as tbl,
            tc.tile_pool(name="pRes", bufs=1) as pres,
            tc.tile_pool(name="dstage", bufs=1, space="DRAM") as dstage,
        ):
            ones_sb = tbl.tile([P, P], BF16, name="ones_sb")
            lnkv_sb = tbl.tile([P, KV_RANK // P], F32, name="lnkv_sb")
            eps_sb = tbl.tile([P, 2], F32, name="eps_sb")
            epskv_sb = eps_sb[:, 0:1]
            epsq_sb = eps_sb[:, 1:2]
            cosq_sb = tbl.tile([P, TQ], BF16, name="cosq_sb")
            sinq_sb = tbl.tile([P, TQ], BF16, name="sinq_sb")

            # SBUF residents
            qnope = pres.tile([P, NH, TQ], BF16, name="qnope")
            qpair = pres.tile([P, 8, TQ], BF16, name="qpair")
            kpair = pres.tile([P, 4, T], BF16, name="kpair")
            kv_latN = pres.tile([P, 4, T], BF16, name="kv_latN")

            # HBM staging for the kv AllGather
            stage_in = dstage.tile([P, 8, TQ], BF16, name="stage_in")
            stage_out = dstage.tile([4, P, 8, TQ], BF16, name="stage_out")

            with tc.tile_pool(name="pBW", bufs=1) as pbw:
                with (
                    tc.tile_pool(name="p2xq", bufs=1) as p2xq,
                    tc.tile_pool(name="p2w", bufs=3) as p2w,
                ):
                    # --------- P0: kv_a on own chunk + AllGather ------------
                    with (
                        tc.tile_pool(name="pKW", bufs=1) as pkw,
                        tc.tile_pool(name="p0", bufs=1) as p0,
                        tc.tile_pool(name="p0s", bufs=2) as p0s,
                        tc.tile_pool(name="p0psL", bufs=4, space="PSUM") as p0psL,
                        tc.tile_pool(name="p0psR", bufs=2, space="PSUM") as p0psR,
                        tc.tile_pool(name="p0ps1", bufs=1, space="PSUM") as p0ps1,
                    ):
                        kva_wt = kva_w.rearrange("(kt p) c -> p kt c", p=P)
                        kvaw_c = []
                        for c in range(4):
                            t_ = pkw.tile([P, 16, 256], BF16, name=f"kvaw_c{c}")
                            kvaw_c.append(t_)
                        xq_c = []
                        for c in range(4):
                            t_ = p2xq.tile([P, 4, TQ], BF16, name=f"xq_c{c}")
                            xq_c.append(t_)

                        # critical path: kvaw_c0 (halved) + the xq tiles
                        nc.sync.dma_start(
                            kvaw_c[0][:, 0:8, :], kva_wt[:, 0:8, 0:256]
                        )
                        nc.sync.dma_start(
                            xq_c[0][:, 0:2, :], xT_t[:, 0:2, 0:TQ]
                        )
                        nc.sync.dma_start(
                            xq_c[0][:, 2:4, :], xT_t[:, 2:4, 0:TQ]
                        )
                        nc.sync.dma_start(
                            kvaw_c[0][:, 8:16, :], kva_wt[:, 8:16, 0:256]
                        )
                        for c in range(1, 4):
                            nc.sync.dma_start(
                                xq_c[c][:], xT_t[:, 4 * c : 4 * c + 4, 0:TQ]
                            )
                        nc.sync.dma_start(ones_sb[:], ones_in[:, :])
                        nc.sync.dma_start(lnkv_sb[:], kva_ln[:, :])
                        nc.sync.dma_start(eps_sb[:], eps_in[:, :])
                        nc.sync.dma_start(cosq_sb[:], cosq[:, :])
                        nc.sync.dma_start(sinq_sb[:], sinq[:, :])
                        qa_wt = qa_w.rearrange("(kt p) c -> p kt c", p=P)
                        qa_wt_pre = []
                        for c in range(1, 4):
                            nc.sync.dma_start(
                                kvaw_c[c][:], kva_wt[:, :, c * 256 : (c + 1) * 256]
                            )
                            # interleave the first q_a weight tiles so the q
                            # path can start the moment P0's matmuls finish
                            w_ = p2w.tile([P, 16, P], BF16, tag="qa_wt")
                            nc.sync.dma_start(
                                w_[:], qa_wt[:, :, (c - 1) * P : c * P]
                            )
                            qa_wt_pre.append(w_)

                        def kvaw_at(k, m):
                            return kvaw_c[m // 2][:, k, (m % 2) * P : (m % 2 + 1) * P]

                        # local stage tiles: normalized latent + paired rope
                        slat = p0.tile([P, 4, TQ], BF16, name="slat")
                        srope = p0.tile([P, 4, TQ], BF16, name="srope")
                        rs_kv = p0.tile([P, 2, 256], F32, name="rs_kv")

                        CW = 256
                        for nch in range(2):
                            chsl = slice(nch * CW, (nch + 1) * CW)
                            sumsq = p0ps1.tile([P, CW], F32, tag="sumsq")
                            raw1 = p0s.tile([P, 2, CW], BF16, tag="kraw1")
                            raw2 = p0s.tile([P, 2, CW], BF16, tag="kraw2")
                            lat_ps = []
                            ksq_pend = []
                            for m in range(8):
                                if m < 4:
                                    ps = p0psL.tile([P, CW], F32, tag="kva_psL")
                                    lat_ps.append(ps)
                                else:
                                    ps = p0psR.tile([P, CW], F32, tag="kva_psR")
                                for k in range(16):
                                    nc.tensor.matmul(
                                        ps[:], kvaw_at(k, m),
                                        xq_c[k // 4][:, k % 4, chsl],
                                        start=(k == 0), stop=(k == 15),
                                    )
                                while ksq_pend:
                                    mm, sq_ = ksq_pend.pop(0)
                                    nc.tensor.matmul(
                                        sumsq[:], ones_sb[:], sq_[:],
                                        start=(mm == 0), stop=(mm == 3),
                                    )
                                if m < 4:
                                    sq = p0s.tile([P, CW], BF16, tag="sq")
                                    nc.scalar.square(sq[:], ps[:])
                                    ksq_pend.append((m, sq))
                                elif m < 6:
                                    nc.scalar.copy(raw1[:, m - 4, :], ps[:])
                                else:
                                    nc.scalar.copy(raw2[:, m - 6, :], ps[:])
                            sqt = p0s.tile([P, CW], F32, tag="sqt")
                            nc.scalar.activation(
                                sqt[:], sumsq[:], AF.Sqrt, bias=epskv_sb[:]
                            )
                            nc.vector.reciprocal(rs_kv[:, nch, :], sqt[:])
                            # normalize straight from PSUM into the stage tile
                            for m in range(4):
                                nc.vector.scalar_tensor_tensor(
                                    slat[:, m, chsl],
                                    lat_ps[m][:],
                                    lnkv_sb[:, m : m + 1],
                                    rs_kv[:, nch, :],
                                    ALU.mult,
                                    ALU.mult,
                                )
                            # rotate rope rows into the paired layout
                            # (cosq/sinq hold this chunk's original angles)
                            for t in range(2):
                                t1 = p0s.tile([P, CW], BF16, tag="rot_a")
                                t2 = p0s.tile([P, CW], BF16, tag="rot_b")
                                t3 = p0s.tile([P, CW], BF16, tag="rot_c")
                                t4 = p0s.tile([P, CW], BF16, tag="rot_d")
                                nc.vector.tensor_tensor(
                                    t1[:], raw1[:, t, :], cosq_sb[:, chsl], ALU.mult
                                )
                                nc.vector.tensor_tensor(
                                    t2[:], raw2[:, t, :], sinq_sb[:, chsl], ALU.mult
                                )
                                nc.vector.tensor_tensor(
                                    t3[:], raw1[:, t, :], sinq_sb[:, chsl], ALU.mult
                                )
                                nc.vector.tensor_tensor(
                                    t4[:], raw2[:, t, :], cosq_sb[:, chsl], ALU.mult
                                )
                                # head kvh=4t+i -> tile kvh//2, base 64*(kvh%2)
                                for i in range(4):
                                    kvh = 4 * t + i
                                    bb = 64 * (kvh % 2)
                                    r = slice(i * 32, i * 32 + 32)
                                    nc.vector.tensor_tensor(
                                        srope[bb : bb + 32, kvh // 2, chsl],
                                        t1[r, :], t2[r, :], ALU.subtract,
                                    )
                                    nc.vector.tensor_tensor(
                                        srope[bb + 32 : bb + 64, kvh // 2, chsl],
                                        t4[r, :], t3[r, :], ALU.add,
                                    )

                        # stage to HBM and AllGather within the batch group
                        # (gpsimd queue: jumps ahead of SP's weight stream)
                        nc.gpsimd.dma_start(stage_in[:, 0:4, :], slat[:])
                        nc.gpsimd.dma_start(stage_in[:, 4:8, :], srope[:])
                        nc.gpsimd.collective_compute(
                            "AllGather",
                            mybir.AluOpType.bypass,
                            replica_groups=GROUPS_CC,
                            ins=[stage_in[:].opt()],
                            outs=[stage_out[:].opt()],
                        )

                    # --------- P2: q path -----------------------------------
                    with (
                        tc.tile_pool(name="p2", bufs=1) as p2,
                        tc.tile_pool(name="p2s", bufs=3) as p2s,
                        tc.tile_pool(name="p2ps", bufs=3, space="PSUM") as p2ps,
                        tc.tile_pool(name="p2ps1", bufs=1, space="PSUM") as p2ps1,
                    ):
                        q_lat = p2.tile([P, Q_RANK // P, TQ], BF16, name="q_lat")
                        rs_q = p2.tile([P, TQ], F32, name="rs_q")

                        # q_a + rmsnorm; sumsq matmul deferred one iteration
                        sumsq = p2ps1.tile([P, TQ], F32, tag="qsumsq")
                        sq_pend = []

                        def flush_sq(last=False):
                            while sq_pend:
                                mm, sq_ = sq_pend.pop(0)
                                nc.tensor.matmul(
                                    sumsq[:], ones_sb[:], sq_[:],
                                    start=(mm == 0), stop=(mm == 11 and last),
                                )

                        for m in range(12):
                            if m < 3:
                                wt = qa_wt_pre[m]
                            else:
                                wt = p2w.tile([P, 16, P], BF16, tag="qa_wt")
                                nc.sync.dma_start(
                                    wt[:], qa_wt[:, :, m * P : (m + 1) * P]
                                )
                            ps = p2ps.tile([P, TQ], F32, tag="qa_ps")
                            for k in range(16):
                                nc.tensor.matmul(
                                    ps[:], wt[:, k, :], xq_c[k // 4][:, k % 4, :],
                                    start=(k == 0), stop=(k == 15),
                                )
                            flush_sq()
                            nc.scalar.copy(q_lat[:, m, :], ps[:])
                            sq = p2s.tile([P, TQ], BF16, tag="qsq")
                            nc.scalar.square(sq[:], ps[:])
                            sq_pend.append((m, sq))
                        flush_sq(last=True)

                        sqt = p2s.tile([P, TQ], F32, tag="qsqt")
                        nc.scalar.activation(
                            sqt[:], sumsq[:], AF.Sqrt, bias=epsq_sb[:]
                        )
                        # rs_q is applied to q_b OUTPUTS (it commutes through
                        # q_b); q_a_ln_w is folded into qb_w rows host-side.
                        nc.vector.reciprocal(rs_q[:], sqt[:])

                        # kv_b weight prefetch (used by P3)
                        kvbw_sb = pbw.tile(
                            [P, 4, NKV * (NOPE + VD)], BF16, name="kvbw_sb"
                        )

                        # q_b: nope into resident qnope; rope raw for rotation
                        with tc.tile_pool(name="p2b", bufs=1) as p2b:
                            qraw1 = p2b.tile([P, 4, TQ], BF16, name="qraw1")
                            qraw2 = p2b.tile([P, 4, TQ], BF16, name="qraw2")
                            for m in range(24):
                                wt = p2w.tile([P, 12, P], BF16, tag="qb_wt")
                                nc.sync.dma_start(
                                    wt[:],
                                    qb_w.rearrange("(kt p) c -> p kt c", p=P)[
                                        :, :, m * P : (m + 1) * P
                                    ],
                                )
                                if m % 12 == 11:  # kv_b chunks mid-stream
                                    c = m // 12
                                    nc.sync.dma_start(
                                        kvbw_sb[:, :, c * 1024 : (c + 1) * 1024],
                                        kvb_w.rearrange("(kt p) c -> p kt c", p=P)[
                                            :, :, c * 1024 : (c + 1) * 1024
                                        ],
                                    )
                                ps = p2ps.tile([P, TQ], F32, tag="qb_ps")
                                for k in range(12):
                                    nc.tensor.matmul(
                                        ps[:], wt[:, k, :], q_lat[:, k, :],
                                        start=(k == 0), stop=(k == 11),
                                    )
                                if m < 16:
                                    nc.vector.tensor_tensor(
                                        qnope[:, m, :], ps[:], rs_q[:], ALU.mult
                                    )
                                elif m < 20:
                                    nc.scalar.copy(qraw1[:, m - 16, :], ps[:])
                                else:
                                    nc.scalar.copy(qraw2[:, m - 20, :], ps[:])

                            # q-rope rotation into the paired layout; rs_q is
                            # folded into the cos/sin tables (k-rope already
                            # consumed the unscaled tables in P0)
                            nc.vector.tensor_tensor(
                                cosq_sb[:], cosq_sb[:], rs_q[:], ALU.mult
                            )
                            nc.vector.tensor_tensor(
                                sinq_sb[:], sinq_sb[:], rs_q[:], ALU.mult
                            )
                            cb = cosq_sb[:, None, :].to_broadcast((P, 4, TQ))
                            sb = sinq_sb[:, None, :].to_broadcast((P, 4, TQ))
                            r1c = p2b.tile([P, 4, TQ], BF16, name="r1c")
                            r2s = p2b.tile([P, 4, TQ], BF16, name="r2s")
                            nc.vector.tensor_tensor(r1c[:], qraw1[:], cb, ALU.mult)
                            nc.vector.tensor_tensor(r2s[:], qraw2[:], sb, ALU.mult)
                            nc.vector.tensor_tensor(qraw1[:], qraw1[:], sb, ALU.mult)
                            nc.vector.tensor_tensor(qraw2[:], qraw2[:], cb, ALU.mult)
                            r1s, r2c = qraw1, qraw2
                            # head h -> tile 2*(h//4)+h%2, base 64*((h//2)%2)
                            for h in range(NH):
                                tq_ = 2 * (h // 4) + h % 2
                                bb = 64 * ((h // 2) % 2)
                                r = slice((h % 4) * 32, (h % 4) * 32 + 32)
                                j = h // 4
                                nc.vector.tensor_tensor(
                                    qpair[bb : bb + 32, tq_, :],
                                    r1c[r, j, :], r2s[r, j, :], ALU.subtract,
                                )
                                nc.vector.tensor_tensor(
                                    qpair[bb + 32 : bb + 64, tq_, :],
                                    r2c[r, j, :], r1s[r, j, :], ALU.add,
                                )

                # --------- P3: attention ------------------------------------
                with (
                    tc.tile_pool(name="pOW", bufs=2) as pow_,
                    tc.tile_pool(name="pA", bufs=1) as pA,
                    tc.tile_pool(name="p3s", bufs=2) as p3s,
                    tc.tile_pool(name="p3q", bufs=4) as p3q,
                    tc.tile_pool(name="p3p", bufs=4) as p3p,
                    tc.tile_pool(name="scps", bufs=4, space="PSUM") as scps,
                    tc.tile_pool(name="atps", bufs=2, space="PSUM") as atps,
                    tc.tile_pool(name="prps", bufs=2, space="PSUM") as prps,
                ):
                    attn_sb = pA.tile([P, NH, TQ], BF16, name="attn_sb")
                    # reload gathered kv via the gpsimd queue (keeps SP's
                    # weight stream and the engine sem-guards unperturbed)
                    for g in range(4):
                        gsl = slice(g * TQ, (g + 1) * TQ)
                        nc.gpsimd.dma_start(kv_latN[:, :, gsl], stage_out[g, :, 0:4, :])
                    for g in range(4):
                        gsl = slice(g * TQ, (g + 1) * TQ)
                        nc.gpsimd.dma_start(kpair[:, :, gsl], stage_out[g, :, 4:8, :])
                    ow_t = o_w.rearrange("(ht p) c -> p ht c", p=P)
                    ow_c = {}

                    def load_ow(n):
                        t_ = pow_.tile([P, NH, 512], BF16, tag="ow")
                        nc.sync.dma_start(t_[:], ow_t[:, :, n * 512 : (n + 1) * 512])
                        ow_c[n] = t_

                    pending = []

                    def finalize(item):
                        dsA, dsB, at, qh = item
                        dn = scps.tile([P, TQ], F32, tag="sc")
                        nc.tensor.matmul(
                            dn[:], ones_sb[:], dsA[:], start=True, stop=False
                        )
                        nc.tensor.matmul(
                            dn[:], ones_sb[:], dsB[:], start=False, stop=True
                        )
                        rec = p3q.tile([P, TQ], F32, tag="rec")
                        nc.vector.reciprocal(rec[:], dn[:])
                        nc.vector.tensor_tensor(
                            attn_sb[:, qh, :], at[:], rec[:], ALU.mult
                        )

                    for hp in range(4):  # kv-head pairs
                        kvh0 = 2 * hp
                        if hp >= 2:  # prefetch first o_w chunks late in P3
                            load_ow(hp - 2)
                        wn = kvbw_sb[:, :, kvh0 * NOPE : (kvh0 + 2) * NOPE]
                        wv = kvbw_sb[
                            :, :, NKV * NOPE + kvh0 * VD : NKV * NOPE + (kvh0 + 2) * VD
                        ]
                        knp = p3s.tile([P, 2, T], BF16, tag="knp")
                        for h2 in range(2):
                            for nch in range(4):
                                ps = prps.tile([P, 512], F32, tag="pr_ps")
                                for k in range(4):
                                    nc.tensor.matmul(
                                        ps[:],
                                        wn[:, k, h2 * P : (h2 + 1) * P],
                                        kv_latN[:, k, nch * 512 : (nch + 1) * 512],
                                        start=(k == 0),
                                        stop=(k == 3),
                                    )
                                nc.vector.tensor_copy(
                                    knp[:, h2, nch * 512 : (nch + 1) * 512], ps[:]
                                )
                        vp = p3s.tile([P, 16, 256], BF16, tag="vp")
                        for mt in range(16):
                            psf = prps.tile([P, 512], F32, tag="pr_ps")
                            ps = psf[:, :256]
                            for k in range(4):
                                nc.tensor.matmul(
                                    ps[:],
                                    kv_latN[:, k, mt * P : (mt + 1) * P],
                                    wv[:, k, :],
                                    start=(k == 0),
                                    stop=(k == 3),
                                )
                            if mt % 2 == 0:
                                nc.scalar.copy(vp[:, mt, :], ps[:])
                            else:
                                nc.vector.tensor_copy(vp[:, mt, :], ps[:])

                        for j4 in range(4):
                            qh = 4 * hp + j4
                            kvh = qh // 2
                            h2 = kvh - kvh0
                            b = 64 * (kvh % 2)
                            tq_ = 2 * (qh // 4) + qh % 2
                            qn = qnope[:, qh, :]
                            qp = qpair[:, tq_, :]
                            krp = kpair[:, hp, :]
                            # two interleaved bf16 partial-sum chains on DVE
                            dsA = p3q.tile([P, TQ], BF16, tag="dsA")
                            dsB = p3q.tile([P, TQ], BF16, tag="dsB")
                            ds = (dsA, dsB)
                            at = atps.tile([P, TQ], F32, tag="at")
                            pts = {}
                            for kt in range(16):
                                sc = scps.tile([P, TQ], F32, tag="sc")
                                nc.tensor.matmul(
                                    sc[:],
                                    knp[:, h2, kt * P : (kt + 1) * P],
                                    qn[:],
                                    start=True,
                                    stop=False,
                                )
                                nc.tensor.matmul(
                                    sc[:],
                                    krp[b : b + 64, kt * P : (kt + 1) * P],
                                    qp[b : b + 64, :],
                                    start=False,
                                    stop=True,
                                )
                                pt = p3p.tile([P, TQ], BF16, tag="probsT")
                                nc.scalar.activation(
                                    pt[:], sc[:], AF.Exp, scale=float(SCALE)
                                )
                                pts[kt] = pt
                                if kt < 2:
                                    nc.vector.tensor_copy(ds[kt][:], pt[:])
                                else:
                                    nc.vector.tensor_tensor(
                                        ds[kt % 2][:], ds[kt % 2][:], pt[:], ALU.add
                                    )
                                if kt > 1:  # PV two stages behind scores
                                    nc.tensor.matmul(
                                        at[:],
                                        vp[:, kt - 2, h2 * P : (h2 + 1) * P],
                                        pts[kt - 2][:],
                                        start=(kt == 2),
                                        stop=False,
                                    )
                                    del pts[kt - 2]
                            for kt in (14, 15):
                                nc.tensor.matmul(
                                    at[:],
                                    vp[:, kt, h2 * P : (h2 + 1) * P],
                                    pts[kt][:],
                                    start=False,
                                    stop=(kt == 15),
                                )
                            pending.append((dsA, dsB, at, qh))
                            if len(pending) == 2:
                                finalize(pending.pop(0))
                    while pending:
                        finalize(pending.pop(0))

                    # --------- P4: o_proj (PSUM reuses prps) ----------------
                    with tc.tile_pool(name="p4s", bufs=2) as p4s:
                        for n in range(4):
                            if n + 2 < 4:  # stream remaining o_w chunks
                                load_ow(n + 2)
                            for mt in range(4):
                                ps = prps.tile([P, 512], F32, tag="pr_ps")
                                for h in range(NH):
                                    nc.tensor.matmul(
                                        ps[:],
                                        attn_sb[:, h, mt * P : (mt + 1) * P],
                                        ow_c[n][:, h, :],
                                        start=(h == 0),
                                        stop=(h == 15),
                                    )
                                st = p4s.tile([P, 512], F32, tag="ost")
                                nc.scalar.copy(st[:], ps[:])
                                nc.sync.dma_start(
                                    out[mt * P : (mt + 1) * P, n * 512 : (n + 1) * 512],
                                    st[:],
                                )

    nc.finalize()
    return nc


def _host_prep(inputs):
    import ml_dtypes

    BF = ml_dtypes.bfloat16

    x = np.asarray(inputs["hidden_states"], dtype=np.float32)
    qa_w = np.asarray(inputs["q_a_w"], np.float32).astype(BF)
    # q_a_ln_w (with the sqrt(Q_RANK) rmsnorm factor) is folded into qb_w rows
    qa_ln_fold = (
        np.asarray(inputs["q_a_ln_w"], np.float64) * math.sqrt(Q_RANK)
    ).astype(np.float32)
    kva_ln = (
        (np.asarray(inputs["kv_a_ln_w"], np.float64) * math.sqrt(KV_RANK))
        .astype(np.float32)
        .reshape(KV_RANK // P, P)
        .T.copy()
    )
    o_w = np.asarray(inputs["o_w"], np.float32).astype(BF)

    qb = (
        np.asarray(inputs["q_b_w"], np.float32) * qa_ln_fold[:, None]
    ).reshape(Q_RANK, NH, HD)
    nope_cols = qb[:, :, :NOPE].reshape(Q_RANK, NH * NOPE)
    rope1 = qb[:, :, NOPE : NOPE + 32].reshape(Q_RANK, 16 * 32)
    rope2 = qb[:, :, NOPE + 32 :].reshape(Q_RANK, 16 * 32)
    qb_w = np.concatenate([nope_cols, rope1, rope2], axis=1).astype(BF)

    kva = np.asarray(inputs["kv_a_w"], np.float32)
    lat = kva[:, :KV_RANK]
    krope = kva[:, KV_RANK:].reshape(HID, NKV, ROPE)
    kr1 = krope[:, :, :32].reshape(HID, NKV * 32)
    kr2 = krope[:, :, 32:].reshape(HID, NKV * 32)
    kva_w = np.concatenate([lat, kr1, kr2], axis=1).astype(BF)

    kvb = np.asarray(inputs["kv_b_w"], np.float32).reshape(KV_RANK, NKV, NOPE + VD)
    knope_cols = kvb[:, :, :NOPE].reshape(KV_RANK, NKV * NOPE)
    v_cols = kvb[:, :, NOPE:].reshape(KV_RANK, NKV * VD)
    kvb_w = np.concatenate([knope_cols, v_cols], axis=1).astype(BF)

    inv_freq = 1.0 / (THETA ** (np.arange(0, ROPE, 2, dtype=np.float32) / ROPE))
    t = np.arange(T, dtype=np.float32)
    freqs = np.outer(t, inv_freq).astype(np.float32)
    cosk_f = np.tile(np.cos(freqs).T, (4, 1)).astype(np.float32)  # [128, T]
    sink_f = np.tile(np.sin(freqs).T, (4, 1)).astype(np.float32)
    ones = np.ones((P, P), np.float32).astype(BF)
    eps2 = np.empty((P, 2), np.float32)
    eps2[:, 0] = EPS * KV_RANK
    eps2[:, 1] = EPS * Q_RANK

    in_maps = []
    for c in range(NCORES):
        b, qc = c // 4, c % 4
        qoff = qc * TQ
        # roll the token axis so this core's query chunk sits at column 0
        xTb = np.ascontiguousarray(np.roll(x[b].T, -qoff, axis=1)).astype(BF)
        in_maps.append(
            {
                "xT": xTb,
                "qa_w": qa_w,
                "qb_w": qb_w,
                "kva_w": kva_w,
                "kva_ln": kva_ln,
                "kvb_w": kvb_w,
                "o_w": o_w,
                "cosq": np.ascontiguousarray(cosk_f[:, qoff : qoff + TQ]).astype(BF),
                "sinq": np.ascontiguousarray(sink_f[:, qoff : qoff + TQ]).astype(BF),
                "ones_in": ones,
                "eps_in": eps2,
            }
        )
    return in_maps


def get_nc():
    if "nc" not in _CACHE:
        _CACHE["nc"] = _build_nc()
    return _CACHE["nc"]


def kernel(**inputs) -> np.ndarray:
    from concourse.bass_utils import run_bass_kernel_spmd

    nc = get_nc()
    in_maps = _host_prep(inputs)
    res = run_bass_kernel_spmd(nc, in_maps, core_ids=list(range(NCORES)))
    _CACHE["last_result"] = res
    outs = [res.results[c]["out"] for c in range(NCORES)]
    full = np.stack(
        [np.concatenate([outs[b * 4 + qc] for qc in range(4)], axis=0) for b in range(B)]
    )
    return full.astype(np.float32)


# revision 64
# speedup vs baseline: 1.5989x; 1.0226x over previous
"""Multi-head latent attention (MLA) TRN2 kernel, v3.

Sharding: batch(2) x query-sequence(4) over 8 cores. Each core:
  - computes kv_a (+rmsnorm+rope) ONLY for its own 512-token chunk and
    AllGathers latent+rope across its 4-core batch group while the whole
    q path runs under the collective
  - computes the Q path for its 512-token query chunk
  - kv_b + full attention for its 512 queries x 2048 keys x 16 heads
  - o_proj for its chunk -> output slice [512, 2048]
Host assembles the 8 slices into [B, T, HID].

Inherited from v2:
  - all matmul operands bf16 (same PE cost as f32r at N>=256, half the
    DMA traffic and SBUF footprint); PSUM stays f32. GPSIMD can't touch
    PSUM and matmuls can't mix f32r/bf16 operands, so the ones-reduction
    tiles (sq, dsA/dsB) are bf16 and all PSUM->SBUF moves sit on ACT/DVE.
  - hidden states arrive rolled so this core's token chunk is at column
    0: kv_a and q_a share the same xq tiles. Key order after the gather
    is the original token order (a permutation, which softmax+PV ignore).
  - q-path rmsnorm scale rs_q commutes through q_b, so it is applied to
    q_b outputs / folded into the rope tables; q_a_ln_w is folded into
    qb_w host-side. Deferred ones-matmuls keep the PE off ACT's tail.
  - rope rotations write 32-row head slices straight into the paired
    layout ([64*(kvh%2)] base so score lhsT/rhs partition bases match).
  - weights prefetched in chunks; gather reload DMAs issue from the
    gpsimd queue so they don't head-of-line-block SP's weight stream.
"""

import math

import numpy as np

B, T, HID = 2, 2048, 2048
NH, NKV = 16, 8
NOPE, ROPE = 128, 64
HD = NOPE + ROPE  # 192
VD = 128
KV_RANK, Q_RANK = 512, 1536
EPS = 1e-6
THETA = 10000.0
NCORES = 8
TQ = B * T // NCORES  # 512 query tokens per core
P = 128
SCALE = 1.0 / math.sqrt(HD)

_CACHE = {}


def _build_nc():
    import concourse.bass as bass  # noqa: F401
    import concourse.mybir as mybir
    from concourse import bacc
    from concourse.tile import TileContext

    F32 = mybir.dt.float32
    BF16 = mybir.dt.bfloat16
    AF = mybir.ActivationFunctionType
    ALU = mybir.AluOpType

    nc = bacc.Bacc(None, target_bir_lowering=False, num_devices=NCORES)

    xT = nc.dram_tensor("xT", [HID, T], BF16, kind="ExternalInput")
    qa_w = nc.dram_tensor("qa_w", [HID, Q_RANK], BF16, kind="ExternalInput")
    qb_w = nc.dram_tensor("qb_w", [Q_RANK, NH * HD], BF16, kind="ExternalInput")
    kva_w = nc.dram_tensor("kva_w", [HID, KV_RANK + NKV * ROPE], BF16, kind="ExternalInput")
    kva_ln = nc.dram_tensor("kva_ln", [P, KV_RANK // P], F32, kind="ExternalInput")
    kvb_w = nc.dram_tensor("kvb_w", [KV_RANK, NKV * (NOPE + VD)], BF16, kind="ExternalInput")
    o_w = nc.dram_tensor("o_w", [NH * VD, HID], BF16, kind="ExternalInput")
    cosq = nc.dram_tensor("cosq", [P, TQ], BF16, kind="ExternalInput")
    sinq = nc.dram_tensor("sinq", [P, TQ], BF16, kind="ExternalInput")
    ones_in = nc.dram_tensor("ones_in", [P, P], BF16, kind="ExternalInput")
    eps_in = nc.dram_tensor("eps_in", [P, 2], F32, kind="ExternalInput")
    out = nc.dram_tensor("out", [TQ, HID], F32, kind="ExternalOutput")

    GROUPS_CC = [[0, 1, 2, 3], [4, 5, 6, 7]]

    xT_t = xT.rearrange("(kt p) t -> p kt t", p=P)  # [128, 16, T]

    with TileContext(nc) as tc:
        with (
            tc.tile_pool(name="tables", bufs=1) as tbl,
            tc.tile_pool(name="pRes", bufs=1) as pres,
            tc.tile_pool(name="dstage", bufs=1, space="DRAM") as dstage,
        ):
            ones_sb = tbl.tile([P, P], BF16, name="ones_sb")
            lnkv_sb = tbl.tile([P, KV_RANK // P], F32, name="lnkv_sb")
            eps_sb = tbl.tile([P, 2], F32, name="eps_sb")
            epskv_sb = eps_sb[:, 0:1]
            epsq_sb = eps_sb[:, 1:2]
            cosq_sb = tbl.tile([P, TQ], BF16, name="cosq_sb")
            sinq_sb = tbl.tile([P, TQ], BF16, name="sinq_sb")

            # SBUF residents
            qnope = pres.tile([P, NH, TQ], BF16, name="qnope")
            qpair = pres.tile([P, 8, TQ], BF16, name="qpair")
            kpair = pres.tile([P, 4, T], BF16, name="kpair")
            kv_latN = pres.tile([P, 4, T], BF16, name="kv_latN")

            # HBM staging for the kv AllGather
            stage_in = dstage.tile([P, 8, TQ], BF16, name="stage_in")
            stage_out = dstage.tile([4, P, 8, TQ], BF16, name="stage_out")

            with tc.tile_pool(name="pBW", bufs=1) as pbw:
                with (
                    tc.tile_pool(name="p2xq", bufs=1) as p2xq,
                    tc.tile_pool(name="p2w", bufs=3) as p2w,
                ):
                    # --------- P0: kv_a on own chunk + AllGather ------------
                    with (
                        tc.tile_pool(name="pKW", bufs=1) as pkw,
                        tc.tile_pool(name="p0", bufs=1) as p0,
                        tc.tile_pool(name="p0s", bufs=2) as p0s,
                        tc.tile_pool(name="p0psL", bufs=4, space="PSUM") as p0psL,
                        tc.tile_pool(name="p0psR", bufs=2, space="PSUM") as p0psR,
                        tc.tile_pool(name="p0ps1", bufs=1, space="PSUM") as p0ps1,
                    ):
                        kva_wt = kva_w.rearrange("(kt p) c -> p kt c", p=P)
                        kvaw_c = []
                        for c in range(4):
                            t_ = pkw.tile([P, 16, 256], BF16, name=f"kvaw_c{c}")
                            kvaw_c.append(t_)
                        xq_c = []
                        for c in range(4):
                            t_ = p2xq.tile([P, 4, TQ], BF16, name=f"xq_c{c}")
                            xq_c.append(t_)

                        # P0 loads in first-use order; kvaw_c2/3 (rope
                        # weights) gate the rotations and thus the collective
                        # launch, so they come before the late-needed tables
                        nc.sync.dma_start(
                            kvaw_c[0][:, 0:2, :], kva_wt[:, 0:2, 0:256]
                        )
                        nc.sync.dma_start(
                            xq_c[0][:, 0:1, :], xT_t[:, 0:1, 0:TQ]
                        )
                        nc.sync.dma_start(
                            kvaw_c[0][:, 2:4, :], kva_wt[:, 2:4, 0:256]
                        )
                        nc.sync.dma_start(
                            xq_c[0][:, 1:2, :], xT_t[:, 1:2, 0:TQ]
                        )
                        nc.sync.dma_start(
                            kvaw_c[0][:, 4:8, :], kva_wt[:, 4:8, 0:256]
                        )
                        nc.sync.dma_start(
                            xq_c[0][:, 2:4, :], xT_t[:, 2:4, 0:TQ]
                        )
                        nc.sync.dma_start(
                            kvaw_c[0][:, 8:16, :], kva_wt[:, 8:16, 0:256]
                        )
                        for c in range(1, 4):
                            nc.sync.dma_start(
                                xq_c[c][:], xT_t[:, 4 * c : 4 * c + 4, 0:TQ]
                            )
                        nc.sync.dma_start(
                            kvaw_c[1][:], kva_wt[:, :, 256:512]
                        )
                        nc.sync.dma_start(ones_sb[:], ones_in[:, :])
                        nc.sync.dma_start(
                            kvaw_c[2][:], kva_wt[:, :, 512:768]
                        )
                        nc.sync.dma_start(
                            kvaw_c[3][:], kva_wt[:, :, 768:1024]
                        )
                        nc.sync.dma_start(lnkv_sb[:], kva_ln[:, :])
                        nc.sync.dma_start(eps_sb[:], eps_in[:, :])
                        nc.sync.dma_start(cosq_sb[:], cosq[:, :])
                        nc.sync.dma_start(sinq_sb[:], sinq[:, :])
                        qa_wt = qa_w.rearrange("(kt p) c -> p kt c", p=P)
                        qa_wt_pre = []
                        for c in range(3):
                            # first q_a weight tiles so the q path can start
                            # the moment P0's matmuls finish
                            w_ = p2w.tile([P, 16, P], BF16, tag="qa_wt")
                            nc.sync.dma_start(
                                w_[:], qa_wt[:, :, c * P : (c + 1) * P]
                            )
                            qa_wt_pre.append(w_)

                        def kvaw_at(k, m):
                            return kvaw_c[m // 2][:, k, (m % 2) * P : (m % 2 + 1) * P]

                        # local stage tiles: normalized latent + paired rope
                        slat = p0.tile([P, 4, TQ], BF16, name="slat")
                        srope = p0.tile([P, 4, TQ], BF16, name="srope")
                        rs_kv = p0.tile([P, TQ], F32, name="rs_kv")

                        # single 512-wide pass over the whole own chunk: one
                        # stt/rotation tail instead of two, earlier launch
                        sumsq = p0ps1.tile([P, TQ], F32, tag="sumsq")
                        raw1 = p0s.tile([P, 2, TQ], BF16, tag="kraw1")
                        raw2 = p0s.tile([P, 2, TQ], BF16, tag="kraw2")
                        lat_ps = []
                        ksq_pend = []
                        ksq_half = []
                        for m in range(8):
                            if m < 4:
                                ps = p0psL.tile([P, TQ], F32, tag="kva_psL")
                                lat_ps.append(ps)
                            else:
                                ps = p0psR.tile([P, TQ], F32, tag="kva_psR")
                            for k in range(16):
                                nc.tensor.matmul(
                                    ps[:], kvaw_at(k, m),
                                    xq_c[k // 4][:, k % 4, :],
                                    start=(k == 0), stop=(k == 15),
                                )
                            while ksq_pend:
                                pp, sq_ = ksq_pend.pop(0)
                                nc.tensor.matmul(
                                    sumsq[:], ones_sb[:], sq_[:],
                                    start=(pp == 0), stop=(pp == 1),
                                )
                            if m < 4:
                                sq = p0s.tile([P, TQ], BF16, tag="sq")
                                nc.scalar.square(sq[:], ps[:])
                                if ksq_half:
                                    # pair-merge on DVE: one ones-matmul per
                                    # two square tiles
                                    nc.vector.tensor_tensor(
                                        sq[:], sq[:], ksq_half.pop()[:], ALU.add
                                    )
                                    ksq_pend.append((m // 2, sq))
                                else:
                                    ksq_half.append(sq)
                            elif m < 6:
                                nc.scalar.copy(raw1[:, m - 4, :], ps[:])
                            else:
                                nc.scalar.copy(raw2[:, m - 6, :], ps[:])
                        sqt = p0s.tile([P, TQ], F32, tag="sqt")
                        nc.scalar.activation(
                            sqt[:], sumsq[:], AF.Sqrt, bias=epskv_sb[:]
                        )
                        nc.vector.reciprocal(rs_kv[:], sqt[:])
                        # normalize straight from PSUM into the stage tile
                        for m in range(4):
                            nc.vector.scalar_tensor_tensor(
                                slat[:, m, :],
                                lat_ps[m][:],
                                lnkv_sb[:, m : m + 1],
                                rs_kv[:],
                                ALU.mult,
                                ALU.mult,
                            )
                        nc.scalar.dma_start(stage_in[:, 0:4, :], slat[:])
                        # rotate rope rows into the paired layout
                        # (cosq/sinq hold this chunk's original angles)
                        for t in range(2):
                            t1 = p0s.tile([P, TQ], BF16, tag="rot_a")
                            t2 = p0s.tile([P, TQ], BF16, tag="rot_b")
                            t3 = p0s.tile([P, TQ], BF16, tag="rot_c")
                            t4 = p0s.tile([P, TQ], BF16, tag="rot_d")
                            nc.vector.tensor_tensor(
                                t1[:], raw1[:, t, :], cosq_sb[:], ALU.mult
                            )
                            nc.vector.tensor_tensor(
                                t2[:], raw2[:, t, :], sinq_sb[:], ALU.mult
                            )
                            nc.vector.tensor_tensor(
                                t3[:], raw1[:, t, :], sinq_sb[:], ALU.mult
                            )
                            nc.vector.tensor_tensor(
                                t4[:], raw2[:, t, :], cosq_sb[:], ALU.mult
                            )
                            # head kvh=4t+i -> tile kvh//2, base 64*(kvh%2)
                            for i in range(4):
                                kvh = 4 * t + i
                                bb = 64 * (kvh % 2)
                                r = slice(i * 32, i * 32 + 32)
                                nc.vector.tensor_tensor(
                                    srope[bb : bb + 32, kvh // 2, :],
                                    t1[r, :], t2[r, :], ALU.subtract,
                                )
                                nc.vector.tensor_tensor(
                                    srope[bb + 32 : bb + 64, kvh // 2, :],
                                    t4[r, :], t3[r, :], ALU.add,
                                )
                        nc.scalar.dma_start(stage_in[:, 4:8, :], srope[:])
                        nc.gpsimd.collective_compute(
                            "AllGather",
                            mybir.AluOpType.bypass,
                            replica_groups=GROUPS_CC,
                            ins=[stage_in[:].opt()],
                            outs=[stage_out[:].opt()],
                        )

                    # --------- P2: q path -----------------------------------
                    with (
                        tc.tile_pool(name="p2", bufs=1) as p2,
                        tc.tile_pool(name="p2s", bufs=3) as p2s,
                        tc.tile_pool(name="p2ps", bufs=3, space="PSUM") as p2ps,
                        tc.tile_pool(name="p2ps1", bufs=1, space="PSUM") as p2ps1,
                    ):
                        q_lat = p2.tile([P, Q_RANK // P, TQ], BF16, name="q_lat")
                        rs_q = p2.tile([P, TQ], F32, name="rs_q")

                        # q_a + rmsnorm; square tiles pair-merged on DVE so
                        # the sumsq reduction is 6 matmuls, deferred so the
                        # PE never waits on the ACT/DVE chain
                        sumsq = p2ps1.tile([P, TQ], F32, tag="qsumsq")
                        sq_pend = []
                        sq_half = []

                        def flush_sq(last=False):
                            while sq_pend:
                                pp, sq_ = sq_pend.pop(0)
                                nc.tensor.matmul(
                                    sumsq[:], ones_sb[:], sq_[:],
                                    start=(pp == 0), stop=(pp == 5 and last),
                                )

                        for m in range(12):
                            if m < 3:
                                wt = qa_wt_pre[m]
                            else:
                                wt = p2w.tile([P, 16, P], BF16, tag="qa_wt")
                                nc.sync.dma_start(
                                    wt[:], qa_wt[:, :, m * P : (m + 1) * P]
                                )
                            ps = p2ps.tile([P, TQ], F32, tag="qa_ps")
                            for k in range(16):
                                nc.tensor.matmul(
                                    ps[:], wt[:, k, :], xq_c[k // 4][:, k % 4, :],
                                    start=(k == 0), stop=(k == 15),
                                )
                            flush_sq()
                            nc.scalar.copy(q_lat[:, m, :], ps[:])
                            sq = p2s.tile([P, TQ], BF16, tag="qsq")
                            nc.scalar.square(sq[:], ps[:])
                            if sq_half:
                                nc.vector.tensor_tensor(
                                    sq[:], sq[:], sq_half.pop()[:], ALU.add
                                )
                                sq_pend.append((m // 2, sq))
                            else:
                                sq_half.append(sq)
                        flush_sq(last=True)

                        sqt = p2s.tile([P, TQ], F32, tag="qsqt")
                        nc.scalar.activation(
                            sqt[:], sumsq[:], AF.Sqrt, bias=epsq_sb[:]
                        )
                        # rs_q is applied to q_b OUTPUTS (it commutes through
                        # q_b); q_a_ln_w is folded into qb_w rows host-side.
                        nc.vector.reciprocal(rs_q[:], sqt[:])

                        # kv_b weight prefetch (used by P3)
                        kvbw_sb = pbw.tile(
                            [P, 4, NKV * (NOPE + VD)], BF16, name="kvbw_sb"
                        )

                        # q_b: nope into resident qnope; rope raw for rotation
                        with tc.tile_pool(name="p2b", bufs=1) as p2b:
                            qraw1 = p2b.tile([P, 4, TQ], BF16, name="qraw1")
                            qraw2 = p2b.tile([P, 4, TQ], BF16, name="qraw2")
                            for m in range(24):
                                wt = p2w.tile([P, 12, P], BF16, tag="qb_wt")
                                nc.sync.dma_start(
                                    wt[:],
                                    qb_w.rearrange("(kt p) c -> p kt c", p=P)[
                                        :, :, m * P : (m + 1) * P
                                    ],
                                )
                                if m % 12 == 11:  # kv_b chunks mid-stream
                                    c = m // 12
                                    nc.sync.dma_start(
                                        kvbw_sb[:, :, c * 1024 : (c + 1) * 1024],
                                        kvb_w.rearrange("(kt p) c -> p kt c", p=P)[
                                            :, :, c * 1024 : (c + 1) * 1024
                                        ],
                                    )
                                ps = p2ps.tile([P, TQ], F32, tag="qb_ps")
                                for k in range(12):
                                    nc.tensor.matmul(
                                        ps[:], wt[:, k, :], q_lat[:, k, :],
                                        start=(k == 0), stop=(k == 11),
                                    )
                                if m < 16:
                                    nc.vector.tensor_tensor(
                                        qnope[:, m, :], ps[:], rs_q[:], ALU.mult
                                    )
                                elif m < 20:
                                    nc.scalar.copy(qraw1[:, m - 16, :], ps[:])
                                else:
                                    nc.scalar.copy(qraw2[:, m - 20, :], ps[:])

                            # q-rope rotation into the paired layout; rs_q is
                            # folded into the cos/sin tables (k-rope already
                            # consumed the unscaled tables in P0)
                            nc.vector.tensor_tensor(
                                cosq_sb[:], cosq_sb[:], rs_q[:], ALU.mult
                            )
                            nc.vector.tensor_tensor(
                                sinq_sb[:], sinq_sb[:], rs_q[:], ALU.mult
                            )
                            cb = cosq_sb[:, None, :].to_broadcast((P, 4, TQ))
                            sb = sinq_sb[:, None, :].to_broadcast((P, 4, TQ))
                            r1c = p2b.tile([P, 4, TQ], BF16, name="r1c")
                            r2s = p2b.tile([P, 4, TQ], BF16, name="r2s")
                            nc.vector.tensor_tensor(r1c[:], qraw1[:], cb, ALU.mult)
                            nc.vector.tensor_tensor(r2s[:], qraw2[:], sb, ALU.mult)
                            nc.vector.tensor_tensor(qraw1[:], qraw1[:], sb, ALU.mult)
                            nc.vector.tensor_tensor(qraw2[:], qraw2[:], cb, ALU.mult)
                            r1s, r2c = qraw1, qraw2
                            # head h -> tile 2*(h//4)+h%2, base 64*((h//2)%2)
                            for h in range(NH):
                                tq_ = 2 * (h // 4) + h % 2
                                bb = 64 * ((h // 2) % 2)
                                r = slice((h % 4) * 32, (h % 4) * 32 + 32)
                                j = h // 4
                                nc.vector.tensor_tensor(
                                    qpair[bb : bb + 32, tq_, :],
                                    r1c[r, j, :], r2s[r, j, :], ALU.subtract,
                                )
                                nc.vector.tensor_tensor(
                                    qpair[bb + 32 : bb + 64, tq_, :],
                                    r2c[r, j, :], r1s[r, j, :], ALU.add,
                                )

                # --------- P3: attention ------------------------------------
                with (
                    tc.tile_pool(name="pOW", bufs=2) as pow_,
                    tc.tile_pool(name="pA", bufs=1) as pA,
                    tc.tile_pool(name="p3s", bufs=2) as p3s,
                    tc.tile_pool(name="p3q", bufs=4) as p3q,
                    tc.tile_pool(name="p3p", bufs=4) as p3p,
                    tc.tile_pool(name="scps", bufs=4, space="PSUM") as scps,
                    tc.tile_pool(name="atps", bufs=2, space="PSUM") as atps,
                    tc.tile_pool(name="prps", bufs=2, space="PSUM") as prps,
                ):
                    attn_sb = pA.tile([P, NH, TQ], BF16, name="attn_sb")
                    # reload gathered kv via the gpsimd queue (a long-latency
                    # wait here must not sit in SP's or ACT's in-order stream)
                    for g in range(4):
                        gsl = slice(g * TQ, (g + 1) * TQ)
                        nc.gpsimd.dma_start(kv_latN[:, :, gsl], stage_out[g, :, 0:4, :])
                    for g in range(4):
                        gsl = slice(g * TQ, (g + 1) * TQ)
                        nc.gpsimd.dma_start(kpair[:, :, gsl], stage_out[g, :, 4:8, :])
                    ow_t = o_w.rearrange("(ht p) c -> p ht c", p=P)
                    ow_c = {}

                    def load_ow(n):
                        t_ = pow_.tile([P, NH, 512], BF16, tag="ow")
                        nc.sync.dma_start(t_[:], ow_t[:, :, n * 512 : (n + 1) * 512])
                        ow_c[n] = t_

                    pending = []

                    def finalize(item):
                        dsA, dsB, at, qh = item
                        # merge the two partial-sum chains on DVE so the
                        # ones-reduction is a single matmul per head
                        nc.vector.tensor_tensor(dsA[:], dsA[:], dsB[:], ALU.add)
                        dn = scps.tile([P, TQ], F32, tag="sc")
                        nc.tensor.matmul(
                            dn[:], ones_sb[:], dsA[:], start=True, stop=True
                        )
                        rec = p3q.tile([P, TQ], F32, tag="rec")
                        nc.vector.reciprocal(rec[:], dn[:])
                        nc.vector.tensor_tensor(
                            attn_sb[:, qh, :], at[:], rec[:], ALU.mult
                        )

                    for hp in range(4):  # kv-head pairs
                        kvh0 = 2 * hp
                        if hp >= 2:  # prefetch first o_w chunks late in P3
                            load_ow(hp - 2)
                        wn = kvbw_sb[:, :, kvh0 * NOPE : (kvh0 + 2) * NOPE]
                        wv = kvbw_sb[
                            :, :, NKV * NOPE + kvh0 * VD : NKV * NOPE + (kvh0 + 2) * VD
                        ]
                        knp = p3s.tile([P, 2, T], BF16, tag="knp")
                        for h2 in range(2):
                            for nch in range(4):
                                ps = prps.tile([P, 512], F32, tag="pr_ps")
                                for k in range(4):
                                    nc.tensor.matmul(
                                        ps[:],
                                        wn[:, k, h2 * P : (h2 + 1) * P],
                                        kv_latN[:, k, nch * 512 : (nch + 1) * 512],
                                        start=(k == 0),
                                        stop=(k == 3),
                                    )
                                nc.vector.tensor_copy(
                                    knp[:, h2, nch * 512 : (nch + 1) * 512], ps[:]
                                )
                        vp = p3s.tile([P, 16, 256], BF16, tag="vp")
                        for mt in range(16):
                            psf = prps.tile([P, 512], F32, tag="pr_ps")
                            ps = psf[:, :256]
                            for k in range(4):
                                nc.tensor.matmul(
                                    ps[:],
                                    kv_latN[:, k, mt * P : (mt + 1) * P],
                                    wv[:, k, :],
                                    start=(k == 0),
                                    stop=(k == 3),
                                )
                            if mt % 2 == 0:
                                nc.scalar.copy(vp[:, mt, :], ps[:])
                            else:
                                nc.vector.tensor_copy(vp[:, mt, :], ps[:])

                        for j4 in range(4):
                            qh = 4 * hp + j4
                            kvh = qh // 2
                            h2 = kvh - kvh0
                            b = 64 * (kvh % 2)
                            tq_ = 2 * (qh // 4) + qh % 2
                            qn = qnope[:, qh, :]
                            qp = qpair[:, tq_, :]
                            krp = kpair[:, hp, :]
                            # two interleaved bf16 partial-sum chains on DVE
                            dsA = p3q.tile([P, TQ], BF16, tag="dsA")
                            dsB = p3q.tile([P, TQ], BF16, tag="dsB")
                            ds = (dsA, dsB)
                            at = atps.tile([P, TQ], F32, tag="at")
                            pts = {}
                            for kt in range(16):
                                sc = scps.tile([P, TQ], F32, tag="sc")
                                nc.tensor.matmul(
                                    sc[:],
                                    knp[:, h2, kt * P : (kt + 1) * P],
                                    qn[:],
                                    start=True,
                                    stop=False,
                                )
                                nc.tensor.matmul(
                                    sc[:],
                                    krp[b : b + 64, kt * P : (kt + 1) * P],
                                    qp[b : b + 64, :],
                                    start=False,
                                    stop=True,
                                )
                                pt = p3p.tile([P, TQ], BF16, tag="probsT")
                                nc.scalar.activation(
                                    pt[:], sc[:], AF.Exp, scale=float(SCALE)
                                )
                                pts[kt] = pt
                                if kt < 2:
                                    nc.vector.tensor_copy(ds[kt][:], pt[:])
                                else:
                                    nc.vector.tensor_tensor(
                                        ds[kt % 2][:], ds[kt % 2][:], pt[:], ALU.add
                                    )
                                if kt > 1:  # PV two stages behind scores
                                    nc.tensor.matmul(
                                        at[:],
                                        vp[:, kt - 2, h2 * P : (h2 + 1) * P],
                                        pts[kt - 2][:],
                                        start=(kt == 2),
                                        stop=False,
                                    )
                                    del pts[kt - 2]
                            for kt in (14, 15):
                                nc.tensor.matmul(
                                    at[:],
                                    vp[:, kt, h2 * P : (h2 + 1) * P],
                                    pts[kt][:],
                                    start=False,
                                    stop=(kt == 15),
                                )
                            pending.append((dsA, dsB, at, qh))
                            if len(pending) == 2:
                                finalize(pending.pop(0))
                    while pending:
                        finalize(pending.pop(0))

                    # --------- P4: o_proj (PSUM reuses prps) ----------------
                    with tc.tile_pool(name="p4s", bufs=2) as p4s:
                        for n in range(4):
                            if n + 2 < 4:  # stream remaining o_w chunks
                                load_ow(n + 2)
                            for mt in range(4):
                                ps = prps.tile([P, 512], F32, tag="pr_ps")
                                for h in range(NH):
                                    nc.tensor.matmul(
                                        ps[:],
                                        attn_sb[:, h, mt * P : (mt + 1) * P],
                                        ow_c[n][:, h, :],
                                        start=(h == 0),
                                        stop=(h == 15),
                                    )
                                st = p4s.tile([P, 512], F32, tag="ost")
                                nc.scalar.copy(st[:], ps[:])
                                nc.sync.dma_start(
                                    out[mt * P : (mt + 1) * P, n * 512 : (n + 1) * 512],
                                    st[:],
                                )

    nc.finalize()
    return nc


def _host_prep(inputs):
    import ml_dtypes

    BF = ml_dtypes.bfloat16

    x = np.asarray(inputs["hidden_states"], dtype=np.float32)
    qa_w = np.asarray(inputs["q_a_w"], np.float32).astype(BF)
    # q_a_ln_w (with the sqrt(Q_RANK) rmsnorm factor) is folded into qb_w rows
    qa_ln_fold = (
        np.asarray(inputs["q_a_ln_w"], np.float64) * math.sqrt(Q_RANK)
    ).astype(np.float32)
    kva_ln = (
        (np.asarray(inputs["kv_a_ln_w"], np.float64) * math.sqrt(KV_RANK))
        .astype(np.float32)
        .reshape(KV_RANK // P, P)
        .T.copy()
    )
    o_w = np.asarray(inputs["o_w"], np.float32).astype(BF)

    qb = (
        np.asarray(inputs["q_b_w"], np.float32) * qa_ln_fold[:, None]
    ).reshape(Q_RANK, NH, HD)
    nope_cols = qb[:, :, :NOPE].reshape(Q_RANK, NH * NOPE)
    rope1 = qb[:, :, NOPE : NOPE + 32].reshape(Q_RANK, 16 * 32)
    rope2 = qb[:, :, NOPE + 32 :].reshape(Q_RANK, 16 * 32)
    qb_w = np.concatenate([nope_cols, rope1, rope2], axis=1).astype(BF)

    kva = np.asarray(inputs["kv_a_w"], np.float32)
    lat = kva[:, :KV_RANK]
    krope = kva[:, KV_RANK:].reshape(HID, NKV, ROPE)
    kr1 = krope[:, :, :32].reshape(HID, NKV * 32)
    kr2 = krope[:, :, 32:].reshape(HID, NKV * 32)
    kva_w = np.concatenate([lat, kr1, kr2], axis=1).astype(BF)

    kvb = np.asarray(inputs["kv_b_w"], np.float32).reshape(KV_RANK, NKV, NOPE + VD)
    knope_cols = kvb[:, :, :NOPE].reshape(KV_RANK, NKV * NOPE)
    v_cols = kvb[:, :, NOPE:].reshape(KV_RANK, NKV * VD)
    kvb_w = np.concatenate([knope_cols, v_cols], axis=1).astype(BF)

    inv_freq = 1.0 / (THETA ** (np.arange(0, ROPE, 2, dtype=np.float32) / ROPE))
    t = np.arange(T, dtype=np.float32)
    freqs = np.outer(t, inv_freq).astype(np.float32)
    cosk_f = np.tile(np.cos(freqs).T, (4, 1)).astype(np.float32)  # [128, T]
    sink_f = np.tile(np.sin(freqs).T, (4, 1)).astype(np.float32)
    ones = np.ones((P, P), np.float32).astype(BF)
    eps2 = np.empty((P, 2), np.float32)
    eps2[:, 0] = EPS * KV_RANK
    eps2[:, 1] = EPS * Q_RANK

    in_maps = []
    for c in range(NCORES):
        b, qc = c // 4, c % 4
        qoff = qc * TQ
        # roll the token axis so this core's query chunk sits at column 0
        xTb = np.ascontiguousarray(np.roll(x[b].T, -qoff, axis=1)).astype(BF)
        in_maps.append(
            {
                "xT": xTb,
                "qa_w": qa_w,
                "qb_w": qb_w,
                "kva_w": kva_w,
                "kva_ln": kva_ln,
                "kvb_w": kvb_w,
                "o_w": o_w,
                "cosq": np.ascontiguousarray(cosk_f[:, qoff : qoff + TQ]).astype(BF),
                "sinq": np.ascontiguousarray(sink_f[:, qoff : qoff + TQ]).astype(BF),
                "ones_in": ones,
                "eps_in": eps2,
            }
        )
    return in_maps


def get_nc():
    if "nc" not in _CACHE:
        _CACHE["nc"] = _build_nc()
    return _CACHE["nc"]


def kernel(**inputs) -> np.ndarray:
    from concourse.bass_utils import run_bass_kernel_spmd

    nc = get_nc()
    in_maps = _host_prep(inputs)
    res = run_bass_kernel_spmd(nc, in_maps, core_ids=list(range(NCORES)))
    _CACHE["last_result"] = res
    outs = [res.results[c]["out"] for c in range(NCORES)]
    full = np.stack(
        [np.concatenate([outs[b * 4 + qc] for qc in range(4)], axis=0) for b in range(B)]
    )
    return full.astype(np.float32)
